# revision 36
# baseline (speedup 1.0000x reference)
"""Distributed Trainium2 kernel for nn_Attention_30262339567666.

Multi-head causal attention with RoPE: B=2, S=2048, HID=2048, NH=16, HD=128.

Sharding: tensor-parallel over heads across 8 cores (2 heads/core).
  - q/k/v column-parallel from replicated hidden states.
  - attention per-core for the local heads; context AllGather'd in fine
    chunks pipelined with attention; o_proj column-parallel.

v3 schedule (per-token-block interleave):
  - step (b,qb): proj(tb) -> attention on that query block for both local
    heads, pumping the next block's projection matmuls as PE fillers. The
    8 context AllGathers fire per half-batch chunk as soon as their ctx
    stores land, pipelining against attention instead of back-loading the
    tail; o_proj blocks become tail fillers as their AG lands.
  - two-deep score pipeline (score kb+2 ahead of pv kb) so PV never waits
    on the ACT exp latency.
  - softmax denominator via GpSimd partition_all_reduce (sum arrives
    broadcast across partitions) — no PE matmuls on the denominator path.
    One fused exp per score tile; the causal mask is a 128-wide in-place
    DVE multiply on the diagonal slice.
  - o_proj is kept out of the PE queue during the last two attention steps
    (they gate the final AG); its backlog then fills the AG-latency window,
    and the final odd-half prefetches are fine-grained so the last o_proj
    matmuls start ~1.5us after the final AG lands.
  - RoPE rotate-half runs entirely on DVE using the duplicated-half
    structure of the cos/sin tables (ACT is reserved for exp).
  - DMA: weights stream on the ACT ring in kt4-consumption order, x blocks
    alternate SP / GpSimd-SWDGE queues, rope tables ride SP, and late ch
    prefetches move to SP so they never head-block exp on the ACT queue.
"""

import sys

sys.path.insert(0, "/opt/trn_rl_repo")

import numpy as np
import ml_dtypes

import concourse.bass as bass
import concourse.tile as tile
from concourse import bacc, bass_isa, mybir
from concourse.bass import _add_dep_helper
from concourse.bass_utils import run_bass_kernel_spmd

# Problem dims
B, S, HID, NH = 2, 2048, 2048, 16
HD = HID // NH           # 128
NC = 8                   # cores
HPC = NH // NC           # heads per core = 2
DL = HPC * HD            # local head dims = 256
T = B * S                # 4096 tokens
NEG = -1e9

BF16 = mybir.dt.bfloat16
F32 = mybir.dt.float32
F32R = mybir.dt.float32r
AF = mybir.ActivationFunctionType

TOK_BLK = 512            # token block for projections / o_proj
N_TB = T // TOK_BLK      # 8
QB = 512                 # query block in attention
KB = 128                 # key tile (partition dim)
KT = HID // 128          # 16 contraction tiles

# AllGather chunking: per (b, m) list of (qb_lo, qb_hi) chunks.
# Each AllGather costs ~20us nearly independent of size (mesh barrier
# machinery dominates), and they serialize on the collective cores — so
# exactly two per (b, m): early half overlaps the pair's own attention
# tail, late half is as small as the fixed cost allows.
AG_CHUNKS = {
    (0, 0): [(0, 2), (2, 4)],
    (0, 1): [(0, 2), (2, 4)],
    (1, 0): [(0, 2), (2, 4)],
    (1, 1): [(0, 2), (2, 4)],
}

LAST_EXEC_NS = None

_CACHE = {}


def _rope_tables():
    """cos/sin tables, transposed to [HD, S], matching reference numerics."""
    inv_freq = 1.0 / (10000.0 ** (np.arange(0, HD, 2, dtype=np.float64) / HD))
    t = np.arange(S, dtype=np.float64)
    freqs = np.outer(t, inv_freq)                 # [S, HD/2]
    emb = np.concatenate([freqs, freqs], axis=-1)  # [S, HD]
    cos = np.cos(emb).astype(np.float32)
    sin = np.sin(emb).astype(np.float32)
    return np.ascontiguousarray(cos.T), np.ascontiguousarray(sin.T)  # [HD, S]


def _build():
    nc = bacc.Bacc("TRN2", target_bir_lowering=False, debug=False,
                   enable_asserts=False, num_devices=NC)

    xT = nc.dram_tensor("xT", [128, N_TB, KT, TOK_BLK], BF16,
                        kind="ExternalInput").ap()
    wqT = nc.dram_tensor("wqT", [128, KT, DL], BF16, kind="ExternalInput").ap()
    wkT = nc.dram_tensor("wkT", [128, KT, DL], BF16, kind="ExternalInput").ap()
    wvT = nc.dram_tensor("wvT", [128, KT, DL], BF16, kind="ExternalInput").ap()
    woT = nc.dram_tensor("woT", [128, KT, DL], BF16, kind="ExternalInput").ap()
    cosT = nc.dram_tensor("cosT", [HD, S], BF16, kind="ExternalInput").ap()
    sinT = nc.dram_tensor("sinT", [HD, S], BF16, kind="ExternalInput").ap()
    masks = nc.dram_tensor("masks", [KB, KB], BF16, kind="ExternalInput").ap()
    out = nc.dram_tensor("out", [DL, T], BF16, kind="ExternalOutput").ap()

    from contextlib import ExitStack
    with tile.TileContext(nc) as tc, ExitStack() as ctx:
        sing = ctx.enter_context(tc.tile_pool(name="sing", bufs=1))
        xpool = ctx.enter_context(tc.tile_pool(name="xpool", bufs=2))
        cpool = ctx.enter_context(tc.tile_pool(name="cpool", bufs=4))
        rpool = ctx.enter_context(tc.tile_pool(name="rpool", bufs=3))
        epool = ctx.enter_context(tc.tile_pool(name="epool", bufs=8))
        spool = ctx.enter_context(tc.tile_pool(name="spool", bufs=2))
        ps_proj = ctx.enter_context(tc.tile_pool(name="ps_proj", bufs=2, space="PSUM"))
        ps_score = ctx.enter_context(tc.tile_pool(name="ps_score", bufs=3, space="PSUM"))
        ps_ctx = ctx.enter_context(tc.tile_pool(name="ps_ctx", bufs=2, space="PSUM"))
        dram = ctx.enter_context(tc.tile_pool(name="dram", bufs=1, space="DRAM"))

        # ---- resident SBUF tensors ----
        wq_sb = sing.tile([128, KT, DL], BF16)
        wk_sb = sing.tile([128, KT, DL], BF16)
        wv_sb = sing.tile([128, KT, DL], BF16)
        wo_sb = sing.tile([128, KT, DL], BF16)
        cos_sb = sing.tile([HD, S], BF16)
        sin_sb = sing.tile([HD, S], BF16)
        mask_sb = sing.tile([KB, KB], BF16)
        qT_sb = sing.tile([128, HPC, T], BF16)
        kT_sb = sing.tile([128, HPC, T], BF16)
        sinneg_sb = sing.tile([HD, S], BF16)
        v_sb = sing.tile([128, HPC, T // 128, HD], BF16)

        # chunked ctx dram tiles
        ctx_loc = {}
        ctx_g = {}
        for (b, m), chunks in AG_CHUNKS.items():
            for ci, (lo, hi) in enumerate(chunks):
                w = (hi - lo) * QB
                ctx_loc[(b, m, ci)] = dram.tile(
                    [HD, w], BF16, name=f"ctx_loc{b}_{m}_{ci}")
                ctx_g[(b, m, ci)] = dram.tile(
                    [NC * HD, w], BF16, addr_space="Shared",
                    name=f"ctx_g{b}_{m}_{ci}")

        def chunk_of(b, m, qb):
            for ci, (lo, hi) in enumerate(AG_CHUNKS[(b, m)]):
                if lo <= qb < hi:
                    return ci, lo
            raise AssertionError

        # ---------------- DMA helpers ----------------
        def load_xblk(tb, fine=False):
            xblk = xpool.tile([128, KT, TOK_BLK], BF16, name="xblk", tag="xblk")
            chunks = ((0, 2), (2, 5), (5, 9), (9, 13), (13, 16)) if fine else \
                     ((0, 4), (4, 8), (8, 12), (12, 16))
            # alternate queues so consecutive blocks load on different rings;
            # odd blocks ride the GpSimd SWDGE queue, which is idle at
            # startup and never competes with exp on the ACT queue.
            eng = nc.sync if tb % 2 == 0 else nc.gpsimd
            for a, bb in chunks:
                eng.dma_start(out=xblk[:, a:bb, :], in_=xT[:, tb, a:bb, :])
            return xblk

        # ---------------- RoPE epilogue ----------------
        H2 = HD // 2  # 64

        def rope_epilogue(psq, dst, pos0, use_act=False):
            # dst = psq*cos + rotate_half(psq)*sin.
            cs = cos_sb[:, pos0:pos0 + TOK_BLK]
            sn = sin_sb[:, pos0:pos0 + TOK_BLK]
            t1 = rpool.tile([128, TOK_BLK], BF16, name="t1", tag="t1")
            t2 = rpool.tile([128, TOK_BLK], BF16, name="t2", tag="t2")
            if use_act:
                # ACT builds rotate_half (only worth it when ACT is idle —
                # outside the attention exp windows).
                nc.scalar.activation(out=t1[0:H2, :], in_=psq[H2:HD, :],
                                     func=AF.Copy, scale=-1.0)
                nc.scalar.activation(out=t1[H2:HD, :], in_=psq[0:H2, :],
                                     func=AF.Copy)
                nc.vector.tensor_mul(t2, psq[:], cs)
                nc.vector.tensor_mul(t1, t1, sn)
                nc.vector.tensor_add(dst, t2, t1)
            else:
                # DVE-only via sin table with the sign of the first half
                # folded in (sinneg rows [0:64] = -sin, rows [64:128] = +sin),
                # exploiting cos/sin row duplication (rows 64:128 == 0:64).
                sneg = sinneg_sb[:, pos0:pos0 + TOK_BLK]
                nc.vector.tensor_mul(t1[0:H2, :], psq[H2:HD, :], sneg[0:H2, :])
                nc.vector.tensor_mul(t1[H2:HD, :], psq[0:H2, :], sneg[H2:HD, :])
                nc.vector.tensor_mul(t2, psq[:], cs)
                nc.vector.tensor_add(dst, t2, t1)

        # ---------------- phase 1 generators ----------------
        def v_chains(tb, xblk, use_act=False):
            for pair in range(2):
                psv = ps_proj.tile([128, 512], F32, name="psv", tag="proj")
                for half in range(2):
                    mt = pair * 2 + half
                    for kt in range(KT):
                        nc.tensor.matmul(
                            psv[:, half * DL:(half + 1) * DL],
                            xblk[:, kt, mt * 128:(mt + 1) * 128],
                            wv_sb[:, kt, :],
                            start=(kt == 0), stop=(kt == KT - 1),
                        )
                        yield
                for half in range(2):
                    mt = pair * 2 + half
                    tt = tb * 4 + mt
                    for m in range(HPC):
                        nc.vector.tensor_copy(
                            out=v_sb[:, m, tt, :],
                            in_=psv[:, half * DL + m * HD: half * DL + (m + 1) * HD])

        def phase1_gen(tb, xblk, use_act=False, kt4=False):
            pos0 = (tb % (S // TOK_BLK)) * TOK_BLK
            t0 = tb * TOK_BLK
            if kt4:
                # 4 parallel q/k chains consuming each kt as it lands —
                # stretches PE consumption across the DMA-bound startup and
                # alternates 4 banks. Borrows score/ctx banks (attention
                # hasn't started yet).
                psqs = [ps_proj.tile([128, TOK_BLK], F32, name="psq0", tag="proj"),
                        ps_proj.tile([128, TOK_BLK], F32, name="psq1", tag="proj"),
                        ps_score.tile([128, TOK_BLK], F32, name="psq2", tag="pss"),
                        ps_ctx.tile([128, TOK_BLK], F32, name="psq3", tag="ctx")]
                specs = [(wq_sb, qT_sb, 0), (wq_sb, qT_sb, 1),
                         (wk_sb, kT_sb, 0), (wk_sb, kT_sb, 1)]
                for kt in range(KT):
                    for ci, (w_sb, dst, m) in enumerate(specs):
                        nc.tensor.matmul(
                            psqs[ci][:],
                            w_sb[:, kt, m * 128:(m + 1) * 128],
                            xblk[:, kt, :],
                            start=(kt == 0), stop=(kt == KT - 1),
                        )
                        yield
                for ci, (w_sb, dst, m) in enumerate(specs):
                    rope_epilogue(psqs[ci], dst[:, m, t0:t0 + TOK_BLK], pos0,
                                  use_act=use_act)
                yield from v_chains(tb, xblk)
                return
            else:
                for w_sb, dst in ((wq_sb, qT_sb), (wk_sb, kT_sb)):
                    for m in range(HPC):
                        psq = ps_proj.tile([128, TOK_BLK], F32, name="psq", tag="proj")
                        for kt in range(KT):
                            nc.tensor.matmul(
                                psq[:],
                                w_sb[:, kt, m * 128:(m + 1) * 128],
                                xblk[:, kt, :],
                                start=(kt == 0), stop=(kt == KT - 1),
                            )
                            yield
                        rope_epilogue(psq, dst[:, m, t0:t0 + TOK_BLK], pos0,
                                      use_act=use_act)
            yield from v_chains(tb, xblk)

        xblks = {}

        def p1_step(tb, kt4=False):
            # lag-1 x prefetch: issue the next block's load partway through
            # this block's q/k chains.
            gen = phase1_gen(tb, xblks.pop(tb), kt4=kt4)
            n = 0
            for _ in gen:
                n += 1
                if n == 24 and tb + 1 < N_TB and (tb + 1) not in xblks:
                    xblks[tb + 1] = load_xblk(tb + 1)
                yield

        # ---------------- filler pump ----------------
        # Two priority queues: p1_q (projection work, consumed first so qkv
        # stays ahead of attention) and op_q (o_proj work, the tail filler).
        # o_proj shares ps_proj banks with p1 chains, but op_q is only
        # reachable once p1_q is empty, so the two never interleave on a
        # PSUM bank.
        p1_q = []   # (tb, gen)
        op_q = []   # gen

        def pump(n=1, op_ok=True):
            done = 0
            while done < n:
                if p1_q:
                    try:
                        next(p1_q[0][1])
                        done += 1
                    except StopIteration:
                        p1_q.pop(0)
                elif op_q and op_ok:
                    try:
                        next(op_q[0])
                        done += 1
                    except StopIteration:
                        op_q.pop(0)
                else:
                    break
            return done

        def finish_p1(tb):
            while p1_q and p1_q[0][0] <= tb:
                try:
                    while True:
                        next(p1_q[0][1])
                except StopIteration:
                    p1_q.pop(0)

        def drain_fillers():
            while pump(64):
                pass

        # ---------------- attention ----------------
        ctx_dmas = {}   # (b, m, qb) -> dma handle

        def attention(b, m, qbs=None, qb_done=None, pump_from=0,
                      pump_n=1, op_ok=True):
            if qbs is None:
                qbs = range(S // QB)
            for qb in qbs:
                q0 = b * S + qb * QB
                nkb = 4 * (qb + 1)
                nquad = nkb // 4
                psc = ps_ctx.tile([128, QB], F32, name="psc", tag="ctx")
                exp_tiles = [None] * nkb
                pa_run = [None]

                def score_exp(kb):
                    j = kb - 4 * qb
                    lo = 128 * j if j > 0 else 0
                    pss = ps_score.tile([128, QB], F32, name="pss", tag="pss")
                    nc.tensor.matmul(
                        pss[:, lo:],
                        kT_sb[:, m, b * S + kb * 128: b * S + (kb + 1) * 128],
                        qT_sb[:, m, q0 + lo:q0 + QB],
                        start=True, stop=True,
                    )
                    expT = epool.tile([128, QB], BF16, name="expT", tag="expT")
                    if lo > 0:
                        nc.vector.memset(expT[:, 0:lo], 0.0)
                    # one ACTIVATE for the whole [lo:] range; the causal mask
                    # is applied in place on just the 128-wide diagonal slice.
                    nc.scalar.activation(out=expT[:, lo:], in_=pss[:, lo:],
                                         func=AF.Exp)
                    if j >= 0:
                        nc.vector.tensor_mul(expT[:, lo:lo + KB],
                                             expT[:, lo:lo + KB], mask_sb[:])
                    exp_tiles[kb] = expT

                def pv(kb):
                    j = kb - 4 * qb
                    lo = 128 * j if j > 0 else 0
                    nc.tensor.matmul(
                        psc[:, lo:],
                        v_sb[:, m, b * 16 + kb, :],
                        exp_tiles[kb][:, lo:],
                        start=(kb == 0), stop=(kb == nkb - 1),
                    )

                def quad_tree(i):
                    # bf16 tree-sum of one quad of exp tiles on DVE, then
                    # accumulate into the running denominator tile.
                    pa = spool.tile([128, QB], BF16, name="pa", tag=f"pa{i}")
                    pb = spool.tile([128, QB], BF16, name="pb", tag="pb")
                    with nc.allow_low_precision(reason="bf16 denom tree sums"):
                        nc.vector.tensor_add(pa, exp_tiles[4 * i],
                                             exp_tiles[4 * i + 1])
                        nc.vector.tensor_add(pb, exp_tiles[4 * i + 2],
                                             exp_tiles[4 * i + 3])
                        nc.vector.tensor_add(pa, pa, pb)
                        if i == 0:
                            pa_run[0] = pa
                        else:
                            nc.vector.tensor_add(pa_run[0], pa_run[0], pa)

                # two-deep score pipeline: score(kb) runs two steps ahead of
                # pv(kb), so pv never waits on the ACT exp latency and the PE
                # p-state stays at max clock through attention.
                score_exp(0)
                if nkb > 1:
                    score_exp(1)
                for kb in range(2, nkb):
                    score_exp(kb)
                    pv(kb - 2)
                    if kb % 4 == 1 and kb >= 5:
                        quad_tree(kb // 4 - 1)
                    if qb >= pump_from:
                        pump(max(pump_n, 4 - qb), op_ok)
                if nkb > 1:
                    pv(nkb - 2)
                pv(nkb - 1)
                quad_tree(nquad - 1)
                # the per-qb epilogue (quad merge -> gpsimd reduce -> recip ->
                # ctx mul -> store) is a ~5us cross-engine latency chain; keep
                # the PE fed through it.
                pump(4 if qb < 3 else 2, op_ok)

                # softmax denominator on GpSimd: partition all-reduce of the
                # merged quad sums gives the per-token sum broadcast across
                # all partitions — no tensor-engine matmuls on this path.
                den = spool.tile([128, QB], F32, name="den", tag="den")
                nc.gpsimd.partition_all_reduce(
                    den, pa_run[0][:], channels=128,
                    reduce_op=bass_isa.ReduceOp.add)
                rec = spool.tile([128, QB], F32, name="rec", tag="rec")
                with nc.allow_low_precision(reason="softmax denom reciprocal"):
                    nc.vector.reciprocal_approx_fast(out=rec, in_=den)
                ctxt = rpool.tile([128, QB], BF16, name="ctxt", tag="ctx_sb")
                nc.vector.tensor_mul(ctxt, psc[:], rec)
                ci, lo = chunk_of(b, m, qb)
                dma = nc.sync.dma_start(
                    out=ctx_loc[(b, m, ci)][:, (qb - lo) * QB:(qb - lo + 1) * QB],
                    in_=ctxt)
                ctx_dmas[(b, m, qb)] = dma
                if qb_done is not None:
                    qb_done(qb)

        def emit_ag(b, m, ci):
            nc.gpsimd.collective_compute(
                "AllGather", mybir.AluOpType.bypass,
                replica_groups=[list(range(NC))],
                ins=[ctx_loc[(b, m, ci)].opt()],
                outs=[ctx_g[(b, m, ci)].opt()])

        # ---------------- phase 2: o_proj ----------------
        c_half = {}

        def prefetch(tb, mh, anchor=None, fine=False):
            b = tb // (S // TOK_BLK)
            qb = tb % (S // TOK_BLK)
            ci, lo = chunk_of(b, mh, qb)
            off = (qb - lo) * TOK_BLK
            ch = cpool.tile([128, KT // 2, TOK_BLK], BF16, name="ch", tag="ch")
            g_r = ctx_g[(b, mh, ci)].rearrange("(t p) n -> p t n", p=128)
            step = 2 if fine else KT // 4
            # late-block prefetches go on the sync queue: the ACT queue is
            # exp-saturated then, and all xblk loads (sync) are already done
            # so a prefetch waiting on its AG sem can't head-block anything
            # that matters.
            eng = nc.sync if tb >= 4 else nc.scalar
            for c0 in range(0, KT // 2, step):
                dma = eng.dma_start(
                    out=ch[:, c0:c0 + step, :],
                    in_=g_r[:, c0:c0 + step, off:off + TOK_BLK])
                if anchor is not None:
                    _add_dep_helper(dma.ins, anchor.ins, sync=True,
                                    reason="prefetch after anchor ctx flow")
            c_half[(tb, mh)] = ch

        def oproj_finish(tb, m, pso, eng=None):
            t0 = tb * TOK_BLK
            osb = spool.tile([128, TOK_BLK], BF16, name="osb", tag="osb")
            if eng == "act":
                nc.scalar.activation(out=osb, in_=pso[:], func=AF.Copy)
            else:
                nc.vector.tensor_copy(out=osb, in_=pso[:])
            nc.sync.dma_start(out=out[m * 128:(m + 1) * 128, t0:t0 + TOK_BLK],
                              in_=osb)

        def oproj_gen(tb, eng=None):
            for m in range(HPC):
                pso = ps_proj.tile([128, TOK_BLK], F32, name="pso", tag="proj")
                i = 0
                for mh in range(2):
                    ch = c_half[(tb, mh)]
                    for j in range(KT // 2):
                        nc.tensor.matmul(
                            pso[:],
                            wo_sb[:, 2 * j + mh, m * 128:(m + 1) * 128],
                            ch[:, j, :],
                            start=(i == 0), stop=(i == KT - 1),
                        )
                        i += 1
                        yield
                oproj_finish(tb, m, pso, eng)

        # ================= emission schedule =================
        # v3: per-token-block interleave. proj(tb) -> attention qb=tb for
        # both local heads (pumping proj(tb+1) as PE filler), AGs fire per
        # half-batch chunk as soon as the ctx is stored, and o_proj blocks
        # become tail fillers as their AG lands. All attention (and all 8
        # AGs) complete well before the PE stream ends, so the tail is pure
        # o_proj instead of an AG-gated trickle.
        #
        # startup: weights+tables on ACT rings, activations on SP rings,
        # chunked in first-use order so the kt4 chains start ~2us in and
        # stream at DMA arrival pace.
        # startup: weights + xblk1 interleaved on the ACT ring in tb0's kt4
        # consumption order; xblk0 + rope tables alone on the SP ring.
        nc.scalar.dma_start(out=wq_sb[:, 0:2, :], in_=wqT[:, 0:2, :])
        nc.scalar.dma_start(out=wk_sb[:, 0:2, :], in_=wkT[:, 0:2, :])
        xblks[0] = load_xblk(0, fine=True)
        nc.sync.dma_start(out=cos_sb, in_=cosT)
        nc.sync.dma_start(out=sin_sb, in_=sinT)
        nc.vector.tensor_scalar_mul(sinneg_sb[0:H2, :], sin_sb[0:H2, :], -1.0)
        nc.vector.tensor_copy(out=sinneg_sb[H2:HD, :], in_=sin_sb[H2:HD, :])
        nc.scalar.dma_start(out=wq_sb[:, 2:5, :], in_=wqT[:, 2:5, :])
        nc.scalar.dma_start(out=wk_sb[:, 2:5, :], in_=wkT[:, 2:5, :])
        xblks[1] = load_xblk(1)   # gpsimd SWDGE queue, parallel to both rings
        nc.scalar.dma_start(out=wq_sb[:, 5:9, :], in_=wqT[:, 5:9, :])
        nc.scalar.dma_start(out=wk_sb[:, 5:9, :], in_=wkT[:, 5:9, :])
        nc.scalar.dma_start(out=wq_sb[:, 9:13, :], in_=wqT[:, 9:13, :])
        nc.scalar.dma_start(out=wk_sb[:, 9:13, :], in_=wkT[:, 9:13, :])
        nc.scalar.dma_start(out=wq_sb[:, 13:, :], in_=wqT[:, 13:, :])
        nc.scalar.dma_start(out=wk_sb[:, 13:, :], in_=wkT[:, 13:, :])
        nc.scalar.dma_start(out=wv_sb[:, 0:8, :], in_=wvT[:, 0:8, :])
        nc.scalar.dma_start(out=wv_sb[:, 8:, :], in_=wvT[:, 8:, :])
        nc.scalar.dma_start(out=mask_sb, in_=masks)

        # tb0 fully before any attention (kt4 stretches the DMA-bound start)
        for _ in p1_step(0, kt4=True):
            pass
        nc.scalar.dma_start(out=wo_sb, in_=woT)
        p1_q.append((1, p1_step(1)))
        p1_queued = 1

        for b, qb in ((0, 0), (0, 1), (0, 2), (0, 3),
                      (1, 0), (1, 1), (1, 2), (1, 3)):
            tb = 4 * b + qb
            finish_p1(tb)
            if tb + 1 < N_TB and p1_queued < tb + 1:
                p1_q.append((tb + 1, p1_step(tb + 1)))
                p1_queued = tb + 1
            # early steps are latency-bound (rope/exp chains, tiny PE work):
            # pump hard to keep the PE fed. The last two steps gate the final
            # AllGather: keep o_proj out of the PE queue there so the
            # attention critical chain isn't delayed — the o_proj backlog
            # then fills the AG-latency window after the loop.
            pump_n = 3 if (b, qb) in ((0, 0), (0, 1)) else 1
            op_ok = (b, qb) not in ((1, 2), (1, 3))
            attention(b, 0, qbs=[qb], pump_n=pump_n, op_ok=op_ok)
            attention(b, 1, qbs=[qb], pump_n=pump_n, op_ok=op_ok)
            if (b, qb) == (0, 1):
                emit_ag(0, 0, 0)
                emit_ag(0, 1, 0)
            elif (b, qb) == (0, 2):
                # blocks 0,1 need (0,*,chunk0): both AGs are in flight
                a = ctx_dmas[(0, 1, 2)]
                for tbp in (0, 1):
                    prefetch(tbp, 0, a)
                    prefetch(tbp, 1, a)
            elif (b, qb) == (0, 3):
                emit_ag(0, 0, 1)
                emit_ag(0, 1, 1)
            elif (b, qb) == (1, 0):
                op_q.append(oproj_gen(0))
                op_q.append(oproj_gen(1))
            elif (b, qb) == (1, 1):
                emit_ag(1, 0, 0)
                emit_ag(1, 1, 0)
                # blocks 2,3 need (0,*,chunk1), landed by now
                a = ctx_dmas[(1, 1, 1)]
                for tbp in (2, 3):
                    prefetch(tbp, 0, a)
                    prefetch(tbp, 1, a)
            elif (b, qb) == (1, 2):
                # blocks 2,3 drain after the attention loop (op_ok is off for
                # the last two steps), when ACT is free — use it for the
                # PSUM->SBUF drains so the DVE stays on attention work.
                op_q.append(oproj_gen(2, eng="act"))
                op_q.append(oproj_gen(3, eng="act"))
                # blocks 4,5 need (1,*,chunk0)
                a = ctx_dmas[(1, 1, 2)]
                for tbp in (4, 5):
                    prefetch(tbp, 0, a)
                    prefetch(tbp, 1, a)
            elif (b, qb) == (1, 3):
                emit_ag(1, 0, 1)
                emit_ag(1, 1, 1)
                # mh=0 halves of the tail blocks come from (1,0,chunk1)
                a = ctx_dmas[(1, 1, 3)]
                prefetch(6, 0, a)
                prefetch(7, 0, a)

        drain_fillers()          # finish o_proj 0-3
        for _ in oproj_gen(4, eng="act"):
            pass
        for _ in oproj_gen(5, eng="act"):
            pass
        # blocks 6,7: split contraction. Even halves (from (1,0,chunk1))
        # run now, borrowing the freed score/ctx banks; the odd halves
        # depend on the final AllGather chunk.
        a6 = ctx_dmas[(1, 1, 3)]
        prefetch(6, 1, a6, fine=True)
        prefetch(7, 1, a6, fine=True)
        pso67 = {
            6: [ps_score.tile([128, TOK_BLK], F32, name=f"pso6_{m}", tag="pss")
                for m in range(HPC)],
            7: [ps_ctx.tile([128, TOK_BLK], F32, name=f"pso7_{m}", tag="ctx")
                for m in range(HPC)],
        }
        for j in range(KT // 2):
            for tb in (6, 7):
                ch = c_half[(tb, 0)]
                for m in range(HPC):
                    nc.tensor.matmul(
                        pso67[tb][m][:],
                        wo_sb[:, 2 * j, m * 128:(m + 1) * 128],
                        ch[:, j, :],
                        start=(j == 0), stop=False,
                    )
        # block-major so block 6's matmuls start as soon as its first fine
        # prefetch chunk lands, while block 7's chunks stream behind.
        for tb in (6, 7):
            ch = c_half[(tb, 1)]
            for j in range(KT // 2):
                for m in range(HPC):
                    nc.tensor.matmul(
                        pso67[tb][m][:],
                        wo_sb[:, 2 * j + 1, m * 128:(m + 1) * 128],
                        ch[:, j, :],
                        start=False, stop=(j == KT // 2 - 1),
                    )
        for tb in (6, 7):
            for m in range(HPC):
                oproj_finish(tb, m, pso67[tb][m], eng="act")

    nc.compile()
    return nc


def kernel(hidden_states, attention_mask, wq, wk, wv, wo):
    global LAST_EXEC_NS
    bf16 = ml_dtypes.bfloat16

    hidden_states = np.asarray(hidden_states, dtype=np.float32)
    wq = np.asarray(wq, dtype=np.float32)
    wk = np.asarray(wk, dtype=np.float32)
    wv = np.asarray(wv, dtype=np.float32)
    wo = np.asarray(wo, dtype=np.float32)

    x = hidden_states.reshape(T, HID)
    # pretiled so every DMA reads contiguous per-partition chunks:
    # xT[p, tb, kt, c] = x[tb*512 + c, kt*128 + p]
    xTt = np.ascontiguousarray(
        x.reshape(N_TB, TOK_BLK, HID // 128, 128).transpose(3, 0, 2, 1)
    ).astype(bf16)
    cosT, sinT = _rope_tables()
    cosT16, sinT16 = cosT.astype(bf16), sinT.astype(bf16)
    k_idx = np.arange(KB)[:, None]
    q_idx = np.arange(KB)[None, :]
    binmask16 = (k_idx <= q_idx).astype(np.float32).astype(bf16)

    def tile_w(w):   # [DL, HID] -> wT tiled [128, KT, DL]
        return np.ascontiguousarray(
            w.T.reshape(HID // 128, 128, DL).transpose(1, 0, 2)).astype(bf16)

    scale = np.float32(1.0 / np.sqrt(HD))
    in_maps = []
    for c in range(NC):
        rows = slice(c * DL, (c + 1) * DL)
        in_maps.append({
            "xT": xTt,
            "wqT": tile_w(wq[rows, :] * scale),
            "wkT": tile_w(wk[rows, :]),
            "wvT": tile_w(wv[rows, :]),
            "woT": tile_w(wo[rows, :]),
            "cosT": cosT16,
            "sinT": sinT16,
            "masks": binmask16,
        })

    if "nc" not in _CACHE:
        _CACHE["nc"] = _build()
    nc = _CACHE["nc"]

    import os
    res = run_bass_kernel_spmd(nc, in_maps, core_ids=list(range(NC)),
                               tmpdir=os.environ.get("BASS_TMPDIR") or None)
    LAST_EXEC_NS = res.exec_time_ns

    outT = np.concatenate([np.asarray(res.results[c]["out"]) for c in range(NC)],
                          axis=0)                          # [HID, T]
    return np.ascontiguousarray(outT.T).reshape(B, S, HID).astype(np.float32)

